# revision 38
# baseline (speedup 1.0000x reference)
"""GNN attention kernel for Trainium2, SPMD over 8 NeuronCores.

Reference computation (per batch b, head h):
    Xp   = X @ Wp[h] + bp[h]                  [N, DH]
    Xc   = Xp @ C[h].T                        [N, DH]
    S    = Xc @ Xp.T                          [N, N]
    attn = tanh(A * S) = A * tanh(S)          (A is binary, tanh(0)=0)
    Xh   = attn @ Xp                          [N, DH]
    out  = relu(concat_h Xh)                  [N, DOUT]

Sharding: data-parallel over batch B=32 -> 4 batches per core. No collectives.

Host-side prep inside kernel(): X and A are transposed per batch and
rounded to bf16 on the CPU, so the device receives XT=[DIN,N] and
AT=[N,N] bf16 and DMAs them straight into the transposed SBUF layouts the
PE needs.  This removes all X/A PE transposes and their PSUM drains.

Per-core dataflow (bf16 on PE, f32 PSUM):
  - Xp  [m, (h k)] = xt.T @ Wp  (+bias on the drain when present)
  - XpT [(2h k), q, n]: PE transpose of Xp tiles.
  - XcT via pair-packed block-diagonal Cpad2: one K=128 matmul per
    (pair, nh) yields both heads' XcT stacked on partitions (rows 0:64 =
    even head, 64:128 = odd head) -- no zero padding needed because the
    score matmuls are row-tiled K=64.
  - Scores: 2x row tiling of the PE (64x128 mode).  Per (pair, i, nh)
    two K=64 matmuls run on PE row-strips 0/1 concurrently: strip u
    contracts head u's 64 features (xpt rows u*64: as lhsT, xct rows
    u*64: as rhs) and writes its own PSUM bank.  A [128, 2, 512] f32
    PSUM tile holds both heads' half-rows; one FD=1024 tanh on ACT
    drains it into att[128, 2u, i, nh*512:] bf16.
  - Mask multiply with AT on DVE (bf16 2x mode) per (u, 4 i-chunks).
  - Aggregate: lhsT = attnT tile, rhs = Xp -> Xh in a [128, 512] f32
    PSUM tile covering 4 j-chunks x (2 heads x 64); ReLU-drain on DVE,
    one DMA per (pair, j-half).

Scheduling: engine queues execute in emission order and the scores/tanh
stream is ACT-gated (~2.0us per (pair, i) slot vs ~0.45us of PE work),
so emission interleaves three streams at slot granularity: scores slots
of batch b, aggregate units of the previous pair, and projection pieces
(xp/xpt/xct) of batch b+1.  Batch 0's projection runs as a prologue that
alternates PSUM pools and ACT/DVE drains (ACT is idle pre-scores).
"""

import os
import sys
import types
from collections import deque
from functools import partial

import ml_dtypes
import numpy as np

import concourse.bass as bass
import concourse.tile as tile
from concourse import bacc, mybir
from concourse.bass_utils import run_bass_kernel_spmd
from concourse.masks import make_identity


def _install_ntff_hook():
    """The image's ``antenv`` lacks ``axon_hooks``; shim it so
    ``run_bass_kernel_spmd(trace=True)`` can capture NTFF profiles through
    the ctypes hook from ``trn_agent_boot``. Degrades silently."""
    if "antenv.axon_hooks" in sys.modules:
        return
    try:
        import antenv  # noqa: F401

        mod = types.ModuleType("antenv.axon_hooks")
        mod._hook = None

        def set_axon_ntff_profile_hook(h):
            mod._hook = h

        def get_axon_ntff_profile_hook():
            return mod._hook

        mod.set_axon_ntff_profile_hook = set_axon_ntff_profile_hook
        mod.get_axon_ntff_profile_hook = get_axon_ntff_profile_hook
        sys.modules["antenv.axon_hooks"] = mod
        from trn_agent_boot.trn_boot import _ntff_profile_via_ctypes

        hook = _ntff_profile_via_ctypes("/opt/axon/libaxon_pjrt.so")
        if hook is not None:
            mod._hook = hook
    except Exception:
        pass


_install_ntff_hook()

B, N, DIN, DOUT, H, DH = 32, 1024, 512, 512, 8, 64
NCORES = 8
BS = B // NCORES          # 4 batches per core
NCH = N // 128            # 8 n/m chunks of 128
DT = DIN // 128           # 4 d tiles
PAIRS = H // 2            # 4 head pairs

F32 = mybir.dt.float32
BF16 = mybir.dt.bfloat16
AF = mybir.ActivationFunctionType

LAST_EXEC_NS = None
LAST_TRACE_DIR = None


def _build(with_bias: bool, n_batch: int = BS):
    nc = bacc.Bacc("TRN2", target_bir_lowering=False, debug=False,
                   num_devices=NCORES)
    XT = nc.dram_tensor("XT", [BS, DIN, N], BF16, kind="ExternalInput").ap()
    AT = nc.dram_tensor("AT", [BS, N, N], BF16, kind="ExternalInput").ap()
    # host-prepared weight layouts (see kernel()): wp staged per d-tile,
    # cpad2 = block-diagonal per-pair C^T
    WPS = nc.dram_tensor("WPS", [128, DT, H, DH], BF16,
                         kind="ExternalInput").ap()
    CPD = nc.dram_tensor("CPD", [128, PAIRS, 128], BF16,
                         kind="ExternalInput").ap()
    bp = None
    if with_bias:
        bp = nc.dram_tensor("bp", [H, DH], F32, kind="ExternalInput").ap()
    OUT = nc.dram_tensor("out", [BS, N, DOUT], BF16,
                         kind="ExternalOutput").ap()

    with tile.TileContext(nc) as tc:
        with (
            tc.tile_pool(name="singles", bufs=1) as singles,
            tc.tile_pool(name="xtp", bufs=2) as xtp,           # xt (dbl buf)
            tc.tile_pool(name="atp", bufs=2) as atp,           # A^T (dbl buf)
            tc.tile_pool(name="xppool", bufs=2) as xppool,     # xp (dbl buf)
            tc.tile_pool(name="xptp", bufs=2) as xptp,         # XpT (dbl buf)
            tc.tile_pool(name="xctp", bufs=2) as xctp,         # XcT (dbl buf)
            tc.tile_pool(name="attnp", bufs=2) as attnp,       # attnT per pair
            tc.tile_pool(name="outt", bufs=4) as outt,         # relu out ring
            tc.tile_pool(name="psT", bufs=2, space="PSUM") as psT,        # 4
            tc.tile_pool(name="psProj", bufs=2, space="PSUM") as psProj,  # 2
            tc.tile_pool(name="psXh", bufs=2, space="PSUM") as psXh,      # 2
        ):
            # ---- one-time setup: direct DMAs of host-prepared weights ----
            identb = singles.tile([128, 128], BF16, name="identb")
            make_identity(nc, identb)

            # warm the tanh ACT table before the scores stream needs it
            warm = singles.tile([128, 16], BF16, name="warm")
            nc.vector.memset(warm, 0.0)
            nc.scalar.activation(warm, warm, AF.Tanh)

            wp_sb = singles.tile([128, DT, H, DH], BF16, name="wp_sb")
            nc.sync.dma_start(out=wp_sb, in_=WPS)
            cpad2 = singles.tile([128, PAIRS, 128], BF16, name="cpad2")
            nc.sync.dma_start(out=cpad2, in_=CPD)

            bias_sb = None
            if with_bias:
                # bp broadcast to all partitions: [128, (h k)] f32 (Xp layout)
                bias_sb = singles.tile([128, H * DH], F32, name="bias_sb")
                bp_flat = bp.rearrange("h k -> (h k)")
                bcast = bass.AP(tensor=bp_flat.tensor, offset=bp_flat.offset,
                                ap=[[0, 128]] + list(bp_flat.ap))
                nc.sync.dma_start(out=bias_sb, in_=bcast)

            # ---- slot-interleaved software pipeline ----
            def proj_start(b):
                """Emit input DMAs for batch b and return its context."""
                xt_sb = xtp.tile([128, DT, N], BF16, tag="xt", name="xt_sb")
                xtr = XT[b].rearrange("(t p) n -> p t n", p=128)
                for n2 in range(2):
                    nc.sync.dma_start(out=xt_sb[:, :, n2 * 512:(n2 + 1) * 512],
                                      in_=xtr[:, :, n2 * 512:(n2 + 1) * 512])
                at_sb = atp.tile([128, NCH, N], BF16, tag="at", name="at_sb")
                return dict(
                    b=b,
                    xt_sb=xt_sb,
                    at_sb=at_sb,
                    xp_sb=xppool.tile([128, NCH, H * DH], BF16, tag="xp",
                                      name="xp_sb"),
                    xpt_sb=xptp.tile([128, PAIRS, N], BF16, tag="xpt",
                                     name="xpt_sb"),
                    xct_sb=xctp.tile([128, PAIRS, N], BF16, tag="xct",
                                     name="xct_sb"),
                    att={},
                )

            def proj_pieces(ctx, deep=False):
                """Deferred-emission pieces computing batch b's projection.
                A^T chunk DMAs ride along as pieces so the 2 MB transfer
                trickles in (bursty DMA SBUF writes slow every engine's
                SBUF streams).  In the batch-0 prologue nothing else runs
                on ACT or psT, so alternate pieces between psProj/psT and
                drains between DVE/ACT to pipeline the serial chain."""
                xt_sb, xp_sb = ctx["xt_sb"], ctx["xp_sb"]
                xpt_sb, xct_sb = ctx["xpt_sb"], ctx["xct_sb"]

                def drain(dst, src_, use_act):
                    if deep and use_act:
                        nc.scalar.activation(dst, src_, AF.Copy)
                    else:
                        nc.vector.tensor_copy(dst, src_)

                def pick(k, shape, dtype, name):
                    if deep and k % 2:
                        return psT.tile(shape, dtype, tag="T", name=name)
                    return psProj.tile(shape, dtype, tag="proj", name=name)

                def xp_piece(j):
                    def f():
                        ps_xp = pick(j, [128, H * DH], F32, "ps_xp")
                        for t in range(DT):
                            nc.tensor.matmul(
                                ps_xp, xt_sb[:, t, j * 128:(j + 1) * 128],
                                wp_sb[:, t, :, :],
                                start=(t == 0), stop=(t == DT - 1))
                        if with_bias:
                            nc.vector.tensor_add(xp_sb[:, j, :], ps_xp,
                                                 bias_sb)
                        else:
                            drain(xp_sb[:, j, :], ps_xp, j % 2 == 0)
                    return f

                def xpt_piece(j):
                    def f():
                        ps_xpt = pick(j, [128, 512], BF16, "ps_xpt")
                        for q in range(PAIRS):
                            nc.tensor.transpose(
                                ps_xpt[:, q * 128:(q + 1) * 128],
                                xp_sb[:, j, q * 128:(q + 1) * 128], identb)
                        drain(xpt_sb[:, :, j * 128:(j + 1) * 128],
                              ps_xpt.rearrange("p (q m) -> p q m", q=PAIRS),
                              j % 2 == 1)
                    return f

                def xct_piece(q, nh):
                    def f():
                        ps_xct = pick(q + nh, [128, 512], F32, "ps_xct")
                        nc.tensor.matmul(ps_xct, cpad2[:, q, :],
                                         xpt_sb[:, q, nh * 512:(nh + 1) * 512],
                                         start=True, stop=True)
                        drain(xct_sb[:, q, nh * 512:(nh + 1) * 512], ps_xct,
                              q % 2 == 0)
                    return f

                pieces = []
                for j in range(4):
                    pieces.append(xp_piece(j))
                for j in range(4):
                    pieces.append(xpt_piece(j))
                if deep:
                    # Shortest path to the first score fill: xct(0,0) only
                    # needs xpt j 0..3.  Pairs 1..3 (and nh=1) fill during
                    # pair 0's first slots (pair q's xct is first read at
                    # slot 8*q).
                    pieces.append(xct_piece(0, 0))
                for j in range(4, NCH):
                    pieces.append(xp_piece(j))
                for j in range(4, NCH):
                    pieces.append(xpt_piece(j))
                if deep:
                    pieces.append(xct_piece(0, 1))
                    leftover = [xct_piece(q, nh)
                                for q in range(1, PAIRS) for nh in range(2)]
                    return pieces, leftover
                for q in range(PAIRS):
                    for nh in range(2):
                        pieces.append(xct_piece(q, nh))
                return pieces

            def at_burst(ctx):
                atr = AT[ctx["b"]].rearrange("(i p) n -> p i n", p=128)
                for i2 in range(2):
                    nc.sync.dma_start(
                        out=ctx["at_sb"][:, 4 * i2:4 * i2 + 4, :],
                        in_=atr[:, 4 * i2:4 * i2 + 4, :])

            pending = {}

            def fill_slot(ctx, q, i):
                """Row-tiled score fills for both heads of pair q, chunk i.
                Strip u (PE rows u*64:) contracts head 2q+u's features and
                writes PSUM bank u of a [128, 1024] tile.  Emitted one slot
                ahead of the tanh so filler work can never delay fills in
                the PE FIFO."""
                xpt_sb, xct_sb = ctx["xpt_sb"], ctx["xct_sb"]
                pss = []
                for nh in range(2):
                    ps = psT.tile([128, 1024], F32, tag="T", name="ps_s")
                    for u in range(2):
                        sl = slice(u * DH, (u + 1) * DH)
                        nc.tensor.matmul(
                            ps[:, u * 512:(u + 1) * 512],
                            xpt_sb[sl, q, i * 128:(i + 1) * 128],
                            xct_sb[sl, q, nh * 512:(nh + 1) * 512],
                            start=True, stop=True)
                    pss.append(ps)
                pending[(id(ctx), q, i)] = pss

            def tanh_slot(ctx, q, i, att):
                pss = pending.pop((id(ctx), q, i))
                for nh in range(2):
                    nc.scalar.activation(att[:, i, nh, :], pss[nh], AF.Tanh)

            def mask_group(ctx, q, i, att):
                at_sb = ctx["at_sb"]
                at4 = at_sb[:, i - 3:i + 1, :].rearrange(
                    "p i (nh n) -> p i nh n", nh=2)
                for u in range(2):
                    sl = slice(u * 512, (u + 1) * 512)
                    nc.vector.tensor_mul(att[:, i - 3:i + 1, :, sl],
                                         att[:, i - 3:i + 1, :, sl],
                                         at4)

            def agg_unit(ctx, q, j2, u, holder, i0=0, i1=NCH,
                         jl0=0, jl1=4):
                """Accumulate Xh for head 2q+u over j-chunks 4*j2..4*j2+3
                and m-chunks i0..i1 into a shared [128, 512] PSUM tile;
                u==1 of the final i-range relu-drains and DMAs the
                [512-node, 128-feat] block."""
                xp_sb = ctx["xp_sb"]
                att = ctx["att"][q]
                b = ctx["b"]
                if u == 0 and i0 == 0 and jl0 == 0:
                    holder[0] = psXh.tile([128, 512], F32, tag="xh",
                                          name="ps_xh")
                ps = holder[0]
                h = 2 * q + u
                for jl in range(jl0, jl1):
                    j = 4 * j2 + jl
                    for i in range(i0, i1):
                        # exactly one start (clears the bank's has_written
                        # bits) and one stop per PSUM tile: later column
                        # groups overwrite-where-unset, then accumulate
                        nc.tensor.matmul(
                            ps[:, jl * 128 + u * DH:jl * 128 + (u + 1) * DH],
                            att[:, i, j // 4,
                                u * 512 + (j % 4) * 128:
                                u * 512 + (j % 4 + 1) * 128],
                            xp_sb[:, i, h * DH:(h + 1) * DH],
                            start=(u == 0 and jl == 0 and i == 0),
                            stop=(u == 1 and jl == 3 and i == NCH - 1),
                            skip_group_check=True)
                if u == 1 and i1 == NCH and jl1 == 4:
                    o = outt.tile([128, 512], BF16, tag="ot", name="o")
                    nc.vector.tensor_scalar_max(o, ps, 0.0)
                    nc.sync.dma_start(
                        out=OUT[b, j2 * 512:(j2 + 1) * 512,
                                q * 128:(q + 1) * 128]
                        .rearrange("(jl p) d -> p jl d", p=128),
                        in_=o.rearrange("p (jl d) -> p jl d", jl=4))

            ctx_next = proj_start(0)
            at_burst(ctx_next)
            prologue, leftover0 = proj_pieces(ctx_next, deep=True)
            for f in prologue:
                f()
            cur = ctx_next
            agg_fill = deque()
            slots = [(q, i) for q in range(PAIRS) for i in range(NCH)]
            prefilled = False
            for b in range(1, n_batch + 1):
                pieces = deque(leftover0)
                leftover0 = []
                if b < n_batch:
                    ctx_next = proj_start(b)
                    pieces.extend(proj_pieces(ctx_next))
                if not prefilled:
                    fill_slot(cur, 0, 0)
                last_pair = None
                for k, (q, i) in enumerate(slots):
                    if k + 1 < len(slots):
                        fill_slot(cur, *slots[k + 1])
                    if k == 8 and b < n_batch:
                        at_burst(ctx_next)
                    if i == 0:
                        # layout: [part, i, nh, (u*512 + n%512)]
                        cur["att"][q] = attnp.tile([128, NCH, 2, 1024], BF16,
                                                   tag="attn", name="att")
                    tanh_slot(cur, q, i, cur["att"][q])
                    if i % 4 == 3:
                        mask_group(cur, q, i, cur["att"][q])
                    if agg_fill:
                        agg_fill.popleft()()
                    if pieces:
                        pieces.popleft()()
                    final_pair = b == n_batch and q == PAIRS - 1
                    if i == 3 and final_pair:
                        # tail shrink: first m-half of the last pair's
                        # aggregation can start right after its mask
                        last_pair = [[None], [None]]
                        agg_fill.extend(
                            partial(agg_unit, cur, q, j2, u, last_pair[j2],
                                    0, 4)
                            for j2 in (0, 1) for u in (0, 1))
                    if i == NCH - 1:
                        if final_pair:
                            agg_fill.extend(
                                partial(agg_unit, cur, q, j2, u,
                                        last_pair[j2], 4, NCH)
                                for j2 in (0, 1) for u in (0, 1))
                        else:
                            holders = [[None], [None]]
                            agg_fill.extend(
                                partial(agg_unit, cur, q, j2, u, holders[j2],
                                        0, NCH, jlh * 2, jlh * 2 + 2)
                                for j2 in (0, 1) for u in (0, 1)
                                for jlh in (0, 1))
                while pieces:
                    pieces.popleft()()
                if b < n_batch:
                    fill_slot(ctx_next, 0, 0)
                    prefilled = True
                cur = ctx_next
            while agg_fill:
                agg_fill.popleft()()

    nc.compile()
    return nc


_CACHED = {}


def _get_nc(with_bias: bool):
    if with_bias not in _CACHED:
        _CACHED[with_bias] = _build(with_bias)
    return _CACHED[with_bias]


def kernel(X, A, Wp, bp, C):
    global LAST_EXEC_NS, LAST_TRACE_DIR
    X = np.asarray(X, dtype=np.float32)
    A = np.asarray(A, dtype=np.float32)
    Wp = np.ascontiguousarray(np.asarray(Wp, dtype=np.float32))
    bp = np.ascontiguousarray(np.asarray(bp, dtype=np.float32))
    C = np.ascontiguousarray(np.asarray(C, dtype=np.float32))

    # Host-side layout prep: per-batch transpose + bf16 round.
    XT = np.ascontiguousarray(
        X.astype(ml_dtypes.bfloat16).transpose(0, 2, 1))
    ATt = np.ascontiguousarray(
        A.astype(ml_dtypes.bfloat16).transpose(0, 2, 1))
    # wp staged per d-tile: WPS[p, t, h, k] = Wp[h, t*128+p, k]
    WPS = np.ascontiguousarray(
        Wp.reshape(H, DT, 128, DH).transpose(2, 1, 0, 3)
    ).astype(ml_dtypes.bfloat16)
    # block-diagonal per-pair C^T: rows u*64+j hold C[2q+u][i, j] in
    # cols u*64+i
    CPD = np.zeros((128, PAIRS, 128), dtype=ml_dtypes.bfloat16)
    for h in range(H):
        u, q = h % 2, h // 2
        CPD[u * DH:(u + 1) * DH, q, u * DH:(u + 1) * DH] = (
            C[h].T.astype(ml_dtypes.bfloat16))

    with_bias = bool(np.any(bp))
    nc = _get_nc(with_bias)

    in_maps = []
    for c in range(NCORES):
        m = {
            "XT": XT[c * BS:(c + 1) * BS],
            "AT": ATt[c * BS:(c + 1) * BS],
            "WPS": WPS,
            "CPD": CPD,
        }
        if with_bias:
            m["bp"] = bp
        in_maps.append(m)

    trace = bool(int(os.environ.get("KERNEL_TRACE", "0")))
    res = run_bass_kernel_spmd(nc, in_maps, core_ids=list(range(NCORES)),
                               trace=trace)
    LAST_EXEC_NS = res.exec_time_ns
    if res.instructions_and_trace is not None:
        LAST_TRACE_DIR = res.instructions_and_trace[1]
    out = np.concatenate([res.results[c]["out"] for c in range(NCORES)], axis=0)
    return out.astype(np.float32)
